# revision 28
# baseline (speedup 1.0000x reference)
"""AttSampler Trainium2 kernel.

out[n,c] = Gy[n] @ data[n,c] @ Gx[n].T  -- separable inverse-CDF attention
sampler (grid gen + bilinear grid_sample), data-parallel over N=8 samples on
8 NeuronCores.

Device pipeline per core (one sample):
  1. Grid gen (attx/atty -> dense 512x512 interp matrices GxT/GyT), replicating
     XLA:CPU's reduction associations bitwise (blocked sum / blocked cumsum).
     Optimized head: fused clip+redistribute updates with PSUM scalar
     pointers, PE-transpose flatten (no DRAM bounce), partition_broadcast
     for row->tile broadcasts, accum_out-fused pcol reductions, tent built
     as relu(1-|p-s|) on the Activation engine. y-axis chain on DVE, x-axis
     on Pool so both resolve concurrently; data loads prefetch from t=0.
  2. Per channel c: two f32r matmul stages with the data / intermediate as
     the stationary operand, which flips layout orientation for free:
        stage1: psum[w_tile, oh] = contraction over h of data with GyT
        stage2: psum[oh_grp, ow] = contraction over w of t1T with GxT
     No transposes anywhere; output comes out in [oh, ow] orientation.
     Rows are interleaved h=4p+q / oh=4p+q across partitions so every
     channel load/store is one contiguous 8KB DMA descriptor per partition.
     Stores issue per q-group so the drain tail is one 2KB-per-partition
     store, not a full channel.
"""

import os
import numpy as np

import concourse.mybir as mybir
import concourse.tile as tile
from concourse import bacc
from concourse.bass_utils import run_bass_kernel_spmd
from concourse.masks import make_identity

N_CORES = 8
C = 32
S = 512          # H = W = out_h = out_w = 512
P = 128          # partitions
NCH = S // P     # 4 chunks per 512 dim
FP32 = mybir.dt.float32
F32R = mybir.dt.float32r

DENSE = 4
ITERS = 5
THR = float(DENSE * S / S)  # 4.0

ALU = mybir.AluOpType
ACT = mybir.ActivationFunctionType

# module-level cache of the built program
_CACHE = {}

# set by run when trace requested (read by test.py)
LAST_EXEC_TIME_NS = None
LAST_RESULTS = None


def _grid_both(nc, tc, sb, psum, attx_dram, atty_dram, gx_sb, gy_sb,
               mid_hook=None):
    """Grid-gen for BOTH axes, x/y stacked in shared tiles so the serial
    dependency chain is paid once. Replicates the reference's XLA:CPU float
    behavior:
      - jnp.sum assoc: seq scan in 16 contiguous windows of 32, then seq
        over the 16 window sums.
      - jnp.cumsum assoc: blocked [32,16] scan (reduce-window lowering).
    All cross-partition movement goes through PE matmuls with 1.0 weights
    (identity / ones / block-indicator products, exact in fp32) or the PE
    transpose path (pure data movement).

    Engine plan: DVE owns the serial y-critical chain; Pool mirrors the
    x-axis work; ACT does the tent Abs/Relu ops and the x flatten copies;
    PE does all transposes/broadcast-feeding matmuls in y-then-x order so
    the main loop's stage-1 matmuls queue right behind gy.
    """
    f = FP32
    ones_col = sb.tile([1, P], f, tag="ones")
    nc.vector.memset(ones_col[:], 1.0)
    ones11 = ones_col[:, 0:1]
    id_sb = sb.tile([P, P], f, tag="id")
    make_identity(nc, id_sb[:])

    # warm the ACT function table before the critical path needs it
    warm = sb.tile([1, 1], f, tag="warm")
    nc.gpsimd.memset(warm[:], 0.0)
    nc.scalar.activation(warm[:], warm[:], ACT.Abs)
    nc.scalar.activation(warm[:], warm[:], ACT.Relu)

    trow = sb.tile([1, S], f, tag="trow")
    nc.gpsimd.iota(trow[:], pattern=[[1, S]], base=1, channel_multiplier=0,
                   allow_small_or_imprecise_dtypes=True)

    # ---- load att in both layouts, stacked (on the SP ring, queued ahead
    # of the channel loads). a16 is partition-padded: x windows on
    # partitions 0:16, y on 32:48, so matmul lhsT bases stay legal. -------
    a16 = sb.tile([64, 32], f, tag="a16")
    nc.gpsimd.memset(a16[:], 0.0)
    nc.sync.dma_start(out=a16[0:16, :],
                      in_=attx_dram.rearrange("(b i) -> b i", b=16))
    nc.sync.dma_start(out=a16[32:48, :],
                      in_=atty_dram.rearrange("(b i) -> b i", b=16))
    a32 = sb.tile([64, 16], f, tag="a32")
    nc.sync.dma_start(out=a32[0:32, :],
                      in_=attx_dram.rearrange("(r j) -> r j", r=32))
    nc.sync.dma_start(out=a32[32:64, :],
                      in_=atty_dram.rearrange("(r j) -> r j", r=32))

    def win_sums(t16, tag):
        """padded [64,32] (x rows 0:16 / y rows 32:48) -> SBUF [1,64] row
        of sequential window-sum prefixes; x total at free 15, y at 47."""
        pr = sb.tile([64, 32], f, tag="sumpr")
        nc.vector.tensor_tensor_scan(pr[:], t16[:], t16[:], 0.0, ALU.add,
                                     ALU.bypass)
        ps_w = psum.tile([1, 64], f, tag="gpsA")
        nc.tensor.matmul(ps_w[:], pr[:, 31:32], id_sb[0:64, 0:64],
                         start=True, stop=True)
        ws = sb.tile([1, 64], f, tag=tag)
        nc.vector.tensor_tensor_scan(ws[:, 0:16], ps_w[:, 0:16],
                                     id_sb[0:1, 0:16], 0.0, ALU.add,
                                     ALU.bypass)
        nc.vector.tensor_tensor_scan(ws[:, 32:48], ps_w[:, 32:48],
                                     id_sb[0:1, 0:16], 0.0, ALU.add,
                                     ALU.bypass)
        # totals as an adjacent [1,2] strided view (x at 15, y at 47)
        return ws[:, 15:48:32]

    def pair_bcast(tag, pair_row):
        """[1,2] (x,y) row -> [64,1] psum column via two accumulating
        mask-row outer products (exact: 1.0/0.0 weights)."""
        ps = psum.tile([64, 1], f, tag="gpsA")
        nc.tensor.matmul(ps[:], bx[tag[0:3]], pair_row[:, 0:1],
                         start=True, stop=False)
        nc.tensor.matmul(ps[:], by[tag[0:3]], pair_row[:, 1:2],
                         start=False, stop=True)
        return ps

    # mask rows for pair_bcast: out[m] = 1.0 on the axis' partition rows
    # b16: x rows 0:16 / y rows 32:48; b32: x rows 0:32 / y rows 32:64
    bx, by = {}, {}
    for name, (x0, x1, y0, y1) in (("b16", (0, 16, 32, 48)),
                                   ("b32", (0, 32, 32, 64))):
        mx = sb.tile([1, 64], f, tag=f"mx_{name}")
        nc.gpsimd.memset(mx[:], 0.0)
        nc.gpsimd.memset(mx[:, x0:x1], 1.0)
        my = sb.tile([1, 64], f, tag=f"my_{name}")
        nc.gpsimd.memset(my[:], 0.0)
        nc.gpsimd.memset(my[:, y0:y1], 1.0)
        bx[name], by[name] = mx[:], my[:]

    # ---- normalize: an = att / sum * S -----------------------------------
    sums = win_sums(a16, "ws_n")
    rrow = sb.tile([1, 2], f, tag="rrow")
    nc.vector.reciprocal(rrow[:], sums)
    r16_ps = pair_bcast("b16_r", rrow)
    r32_ps = pair_bcast("b32_r", rrow)
    an16 = sb.tile([64, 32], f, tag="an16")
    nc.vector.tensor_scalar(an16[:], a16[:], r16_ps[:], float(S),
                            op0=ALU.mult, op1=ALU.mult)
    an32 = sb.tile([64, 16], f, tag="an32")
    nc.vector.tensor_scalar(an32[:], a32[:], r32_ps[:], float(S),
                            op0=ALU.mult, op1=ALU.mult)

    # ---- 5 redistribute iterations ---------------------------------------
    # carried state: c16 = min(att, thr) (clipped layout for the sums) and
    # an32 = att (cumsum layout). Updates fuse to one op each, bitwise equal
    # to the reference's {clip; sum; redistribute} order. Ping-pong tags:
    # each update reads its previous tile (bufs=1 per tag).
    c16 = sb.tile([64, 32], f, tag="c16i")
    nc.vector.tensor_scalar(c16[:], an16[:], THR, None, op0=ALU.min)
    for it in range(ITERS):
        sums = win_sums(c16, "ws_i")
        drow = sb.tile([1, 2], f, tag="drow")
        nc.vector.tensor_scalar(drow[:], sums, -1.0 / S, 1.0,
                                op0=ALU.mult, op1=ALU.add)
        d16_ps = pair_bcast("b16_d", drow)
        d32_ps = pair_bcast("b32_d", drow)
        if it < ITERS - 1:
            c16n = sb.tile([64, 32], f, tag=f"c16{it % 2}")
            nc.vector.tensor_scalar(c16n[:], c16[:], d16_ps[:], THR,
                                    op0=ALU.add, op1=ALU.min)
            c16 = c16n
        an32n = sb.tile([64, 16], f, tag=f"an32{it % 2}")
        nc.vector.tensor_scalar(an32n[:], an32[:], THR, d32_ps[:],
                                op0=ALU.min, op1=ALU.add)
        an32 = an32n

    # ---- XLA cumsum replica (blocked [32,16] per axis, stacked [64,16]) --
    pr16 = sb.tile([64, 16], f, tag="pr16")
    nc.vector.tensor_tensor_scan(pr16[:], an32[:], an32[:], 0.0, ALU.add,
                                 ALU.bypass)
    ps_b = psum.tile([1, 64], f, tag="gpsA")
    nc.tensor.matmul(ps_b[:, 0:32], pr16[0:32, 15:16], id_sb[0:32, 0:32],
                     start=True, stop=True)
    nc.tensor.matmul(ps_b[:, 32:64], pr16[32:64, 15:16], id_sb[32:64, 32:64],
                     start=True, stop=True)
    # two-stage sequential scan over the 32 block sums per axis (XLA order)
    incl = sb.tile([1, 64], f, tag="incl")
    for base in (0, 32):
        nc.vector.tensor_tensor_scan(incl[:, base:base + 16],
                                     ps_b[:, base:base + 16],
                                     id_sb[0:1, 0:16], 0.0, ALU.add,
                                     ALU.bypass)
        nc.vector.tensor_tensor_scan(incl[:, base + 16:base + 32],
                                     ps_b[:, base + 16:base + 32],
                                     id_sb[0:1, 0:16], 0.0, ALU.add,
                                     ALU.bypass)
        nc.vector.tensor_scalar(incl[:, base + 16:base + 32],
                                incl[:, base + 16:base + 32],
                                incl[:, base + 15:base + 16], None,
                                op0=ALU.add)
    excl = sb.tile([1, 64], f, tag="excl")
    nc.vector.memset(excl[:], 0.0)
    nc.vector.tensor_copy(excl[:, 1:32], incl[:, 0:31])
    nc.vector.tensor_copy(excl[:, 33:64], incl[:, 32:63])
    ecol_ps = psum.tile([64, 1], f, tag="gpsA")
    nc.tensor.matmul(ecol_ps[:], excl[:], ones11, start=True, stop=True)
    csum2 = sb.tile([64, 16], f, tag="csum2")
    nc.vector.tensor_scalar(csum2[:], pr16[:], ecol_ps[:], None, op0=ALU.add)

    # ---- flatten csum [64,16] -> guarded row via PE transpose ------------
    # crow layout: [0]=0, x csum at 1..512, [513]=0, y csum at 514..1025.
    # c1row (csum shifted by one, zero-padded) is then just the off-by-one
    # view of the same buffer.
    ps_t = psum.tile([16, 64], f, tag="gpsA")
    nc.tensor.transpose(ps_t[:], csum2[:], id_sb[0:64, 0:64])
    # separate per-axis row tiles so the two engines' strided copies never
    # WAW-serialize against each other
    crow_x = sb.tile([1, S + 1], f, tag="crowx")
    crow_y = sb.tile([1, S + 1], f, tag="crowy")
    nc.gpsimd.memset(crow_x[0:1, 0:1], 0.0)
    nc.vector.memset(crow_y[0:1, 0:1], 0.0)
    vx = crow_x[0:1, 1:513].rearrange("a (r j) -> a r j", r=32)
    vy = crow_y[0:1, 1:513].rearrange("a (r j) -> a r j", r=32)
    for j in range(16):
        nc.vector.tensor_copy(vy[:, :, j], ps_t[j:j + 1, 32:64])
    for j in range(16):
        nc.scalar.copy(vx[:, :, j], ps_t[j:j + 1, 0:32])

    # per-axis phases; y first (stage-1 needs gy; mid_hook lets the caller
    # emit the first stage-1 channels into the PE queue before the x-phase
    # matmuls, so the main loop starts at gy-ready)
    ones_r = sb.tile([P, 1], F32R, tag="ones_r")
    nc.vector.memset(ones_r[:], 1.0)

    def axis_phase(g_sb, crow, veng, sfx):
        c1v = crow[0:1, 0:S]          # csum[s-1], zero-guarded
        cv = crow[0:1, 1:S + 1]       # csum[s]

        # rows: dd = max(csum - csum_sm1, 1e-8); rd = 1/dd
        dd = sb.tile([1, S], f, tag=f"dd{sfx}")
        veng.tensor_tensor(dd[:], cv, c1v, op=ALU.subtract)
        veng.tensor_scalar(dd[:], dd[:], 1e-8, None, op0=ALU.max)
        rd = sb.tile([1, S], f, tag=f"rd{sfx}")
        nc.vector.reciprocal(rd[:], dd[:])

        # targets: tgt = step * (1..512), step = csum[-1]/512, broadcast
        stp = sb.tile([1, 1], f, tag=f"stp{sfx}")
        veng.tensor_scalar(stp[:], crow[0:1, S:S + 1],
                           1.0 / S, None, op0=ALU.mult)
        tgt = sb.tile([1, S], f, tag=f"tgt{sfx}")
        veng.tensor_scalar(tgt[:], trow[:], stp[:], None, op0=ALU.mult)
        tgtb = sb.tile([P, S], f, tag=f"tgtb{sfx}")
        nc.gpsimd.partition_broadcast(tgtb[:], tgt[:])

        # csum_sm1 / rd as per-partition scalar columns [128, 2*NCH]
        # (s = 128k + p), via tiny exact ones-product matmuls
        gcol_ps = psum.tile([P, 2 * NCH], f, tag="gpsA")
        for k in range(NCH):
            nc.tensor.matmul(gcol_ps[:, k:k + 1],
                             c1v[:, k * P:(k + 1) * P], ones11,
                             start=True, stop=True)
            nc.tensor.matmul(gcol_ps[:, NCH + k:NCH + k + 1],
                             rd[:, k * P:(k + 1) * P], ones11,
                             start=True, stop=True)
        gcol = sb.tile([P, 2 * NCH], f, tag=f"gcol{sfx}")
        veng.tensor_copy(gcol[:], gcol_ps[:])

        # p row = sum_s clip((tgt[t]-csum_sm1[s])*rd[s], 0, 1) = j + frac,
        # accumulated over the 4 s-chunks by exact ones-matmuls on PE
        prow_ps = psum.tile([1, S], f, tag="gpsR")
        for k in range(NCH):
            t2 = sb.tile([P, S], F32R, tag=f"pt{sfx}{k % 2}")
            veng.tensor_scalar(t2[:], tgtb[:], gcol[:, k:k + 1],
                               gcol[:, NCH + k:NCH + k + 1],
                               op0=ALU.subtract, op1=ALU.mult)
            veng.tensor_scalar(t2[:], t2[:], 0.0, 1.0, op0=ALU.max,
                               op1=ALU.min)
            nc.tensor.matmul(prow_ps[:], ones_r[:], t2[:],
                             start=(k == 0), stop=(k == NCH - 1))

        # p_img = p * 511/512 (matches the reference's coord chain to ~1
        # ulp; the inverse-CDF p is scale-exact either way), broadcast
        p_img = sb.tile([1, S], f, tag=f"pimg{sfx}")
        veng.tensor_scalar(p_img[:], prow_ps[:], float(S - 1) / S, None,
                           op0=ALU.mult)
        pb = sb.tile([P, S], f, tag=f"pb{sfx}")
        nc.gpsimd.partition_broadcast(pb[:], p_img[:])

        # tent build: G[s,t] = relu(1 - |p[t] - s|), bitwise equal to the
        # reference's clip(p-s+1,0,1)-clip(p-s,0,1) for the 2-nonzero rows.
        # gx in blocked layout s = 128k+p (stage-2 k-chunks);
        # gy in interleaved layout s = 4p+q (8KB-descriptor loads)
        for k in range(NCH):
            scol = sb.tile([P, 1], f, tag=f"scol{sfx}{k % 2}")
            if g_sb is gx_sb:
                nc.gpsimd.iota(scol[:], pattern=[[0, 1]], base=k * P,
                               channel_multiplier=1,
                               allow_small_or_imprecise_dtypes=True)
            else:
                nc.gpsimd.iota(scol[:], pattern=[[0, 1]], base=k,
                               channel_multiplier=NCH,
                               allow_small_or_imprecise_dtypes=True)
            teng = veng if (veng is not nc.vector or k < 2) else nc.gpsimd
            t0 = sb.tile([P, S], f, tag=f"g0{sfx}{k % 2}")
            teng.tensor_scalar(t0[:], pb[:], scol[:], None, op0=ALU.subtract)
            ab = sb.tile([P, S], f, tag=f"ga{sfx}{k % 2}")
            if teng is nc.vector:
                teng.scalar_tensor_tensor(ab[:], t0[:], -1.0, t0[:],
                                          op0=ALU.mult, op1=ALU.max)
            else:
                nc.scalar.activation(ab[:], t0[:], ACT.Abs)
            nc.scalar.activation(g_sb[:, k, :], ab[:], ACT.Relu, bias=1.0,
                                 scale=-1.0)

    axis_phase(gy_sb, crow_y, nc.vector, "y")
    if mid_hook is not None:
        mid_hook()
    axis_phase(gx_sb, crow_x, nc.gpsimd, "x")

    return {"crow_x": crow_x[:], "crow_y": crow_y[:]}


def _build_program():
    nc = bacc.Bacc("TRN2", target_bir_lowering=False, debug=False,
                   num_devices=N_CORES)

    data_h = nc.dram_tensor("data", [C, S, S], FP32, kind="ExternalInput")
    attx_h = nc.dram_tensor("attx", [S], FP32, kind="ExternalInput")
    atty_h = nc.dram_tensor("atty", [S], FP32, kind="ExternalInput")
    out_h = nc.dram_tensor("out", [C, S, S], FP32, kind="ExternalOutput")

    with tile.TileContext(nc) as tc:
        from contextlib import ExitStack
        with ExitStack() as ctx:
            # long-lived pools FIRST so grid temporaries never alias them
            # (aliasing would stall the prefetched loads on grid-gen)
            gpool = ctx.enter_context(tc.tile_pool(name="g_sb", bufs=1))
            gx_sb = gpool.tile([P, NCH, S], F32R, tag="gx")
            gy_sb = gpool.tile([P, NCH, S], F32R, tag="gy")

            dpool = ctx.enter_context(tc.tile_pool(name="dtile", bufs=6))
            tpool = ctx.enter_context(tc.tile_pool(name="t1t", bufs=4))
            opool = ctx.enter_context(tc.tile_pool(name="osb", bufs=3))
            ps1 = ctx.enter_context(
                tc.tile_pool(name="ps1", bufs=3, space="PSUM"))
            ps2 = ctx.enter_context(
                tc.tile_pool(name="ps2", bufs=3, space="PSUM"))

            PIPE = 3  # stage-1 lookahead channels emitted before the x-grid

            def emit_stage1(c):
                # interleaved row layout: partition p holds rows 4p..4p+3
                # (one contiguous 8KB descriptor per partition)
                dt = dpool.tile([P, NCH, S], F32R, tag="d")
                nc.sync.dma_start(
                    out=dt[:],
                    in_=data_h[c].rearrange("(p q) w -> p q w", p=P).bitcast(F32R))
                # stage 1: t1T[w, oh] = sum_h data[h, w] * GyT[h, oh],
                # contraction split by q = h%4 (gy is s=4p+q interleaved)
                t1 = tpool.tile([P, NCH, S], F32R, tag="t1")
                for m in range(NCH):
                    pt = ps1.tile([P, S], FP32, tag="ps1")
                    for q in range(NCH):
                        nc.tensor.matmul(pt[:],
                                         dt[:, q, m * P:(m + 1) * P],
                                         gy_sb[:, q, :],
                                         start=(q == 0), stop=(q == NCH - 1))
                    nc.vector.tensor_copy(t1[:, m, :], pt[:])
                return t1

            def emit_stage2(c, t1):
                # stage 2: out[oh, ow] = sum_w t1T[w, oh] * GxT[w, ow];
                # m-groups pick oh = 4p+q (stride-4 slice) so each q-group
                # store is one contiguous 2KB descriptor per partition
                osb = opool.tile([P, NCH, S], FP32, tag="o")
                ov = out_h[c].rearrange("(p q) w -> p q w", p=P)
                for q in range(NCH):
                    pt = ps2.tile([P, S], FP32, tag="ps2")
                    for k in range(NCH):
                        nc.tensor.matmul(pt[:],
                                         t1[:, k, q::NCH],
                                         gx_sb[:, k, :],
                                         start=(k == 0), stop=(k == NCH - 1))
                    nc.scalar.copy(osb[:, q, :], pt[:])
                    # store on the SP HWDGE ring: loads always run ahead of
                    # the matching store there, and keeping stores off the
                    # ACT ring stops them serializing against the psum
                    # copies; per-q stores shrink the drain tail
                    nc.sync.dma_start(out=ov[:, q, :], in_=osb[:, q, :])

            t1s = {}

            def mid_hook():
                for c in range(PIPE):
                    t1s[c] = emit_stage1(c)

            with ExitStack() as gctx:
                sb = gctx.enter_context(tc.tile_pool(name="grid_sb", bufs=1))
                psum_g = gctx.enter_context(
                    tc.tile_pool(name="grid_ps", bufs=1, space="PSUM"))
                dbg = _grid_both(nc, tc, sb, psum_g, attx_h[:], atty_h[:],
                                 gx_sb, gy_sb, mid_hook=mid_hook)
                if os.environ.get("ATT_DEBUG_GRID"):
                    dbg_gy = nc.dram_tensor("dbg_gy", [P, NCH, S], FP32,
                                            kind="ExternalOutput")
                    dbg_gx = nc.dram_tensor("dbg_gx", [P, NCH, S], FP32,
                                            kind="ExternalOutput")
                    dbg_crow = nc.dram_tensor("dbg_crow", [2 * S + 2], FP32,
                                              kind="ExternalOutput")
                    nc.scalar.dma_start(out=dbg_gy[:],
                                        in_=gy_sb[:].bitcast(FP32))
                    nc.scalar.dma_start(out=dbg_gx[:],
                                        in_=gx_sb[:].bitcast(FP32))
                    nc.scalar.dma_start(
                        out=dbg_crow[0:S + 1].rearrange("(a s) -> a s", a=1),
                        in_=dbg["crow_x"])
                    nc.scalar.dma_start(
                        out=dbg_crow[S + 1:2 * S + 2].rearrange(
                            "(a s) -> a s", a=1),
                        in_=dbg["crow_y"])

            for c in range(C):
                if c + PIPE < C:
                    t1s[c + PIPE] = emit_stage1(c + PIPE)
                emit_stage2(c, t1s.pop(c))

    nc.compile()
    return nc


def _get_program():
    key = "nc"
    if key not in _CACHE:
        _CACHE[key] = _build_program()
    return _CACHE[key]


def kernel(data, attx, atty):
    global LAST_EXEC_TIME_NS, LAST_RESULTS
    data = np.ascontiguousarray(data, dtype=np.float32)
    attx = np.ascontiguousarray(attx, dtype=np.float32)
    atty = np.ascontiguousarray(atty, dtype=np.float32)
    N = data.shape[0]
    assert N == N_CORES

    nc = _get_program()
    in_maps = [
        {
            "data": data[i],
            "attx": attx[i].reshape(S),
            "atty": atty[i].reshape(S),
        }
        for i in range(N)
    ]
    trace = bool(int(os.environ.get("ATT_KERNEL_TRACE", "0")))
    try:
        res = run_bass_kernel_spmd(nc, in_maps, list(range(N_CORES)),
                                   trace=trace)
    except ModuleNotFoundError:
        # NTFF profile hook unavailable in this environment
        res = run_bass_kernel_spmd(nc, in_maps, list(range(N_CORES)),
                                   trace=False)
    LAST_EXEC_TIME_NS = res.exec_time_ns
    LAST_RESULTS = res
    out = np.stack([res.results[i]["out"] for i in range(N)], axis=0)
    return out


# revision 29
# speedup vs baseline: 1.0024x; 1.0024x over previous
"""AttSampler Trainium2 kernel.

out[n,c] = Gy[n] @ data[n,c] @ Gx[n].T  -- separable inverse-CDF attention
sampler (grid gen + bilinear grid_sample), data-parallel over N=8 samples on
8 NeuronCores.

Device pipeline per core (one sample):
  1. Grid gen (attx/atty -> dense 512x512 interp matrices GxT/GyT), replicating
     XLA:CPU's reduction associations bitwise (blocked sum / blocked cumsum).
     Optimized head: fused clip+redistribute updates with PSUM scalar
     pointers, PE-transpose flatten (no DRAM bounce), partition_broadcast
     for row->tile broadcasts, accum_out-fused pcol reductions, tent built
     as relu(1-|p-s|) on the Activation engine. y-axis chain on DVE, x-axis
     on Pool so both resolve concurrently; data loads prefetch from t=0.
  2. Per channel c: two f32r matmul stages with the data / intermediate as
     the stationary operand, which flips layout orientation for free:
        stage1: psum[w_tile, oh] = contraction over h of data with GyT
        stage2: psum[oh_grp, ow] = contraction over w of t1T with GxT
     No transposes anywhere; output comes out in [oh, ow] orientation.
     Rows are interleaved h=4p+q / oh=4p+q across partitions so every
     channel load/store is one contiguous 8KB DMA descriptor per partition.
     Stores issue per q-group so the drain tail is one 2KB-per-partition
     store, not a full channel.
"""

import os
import numpy as np

import concourse.mybir as mybir
import concourse.tile as tile
from concourse import bacc
from concourse.bass_utils import run_bass_kernel_spmd
from concourse.masks import make_identity

N_CORES = 8
C = 32
S = 512          # H = W = out_h = out_w = 512
P = 128          # partitions
NCH = S // P     # 4 chunks per 512 dim
FP32 = mybir.dt.float32
F32R = mybir.dt.float32r

DENSE = 4
ITERS = 5
THR = float(DENSE * S / S)  # 4.0

ALU = mybir.AluOpType
ACT = mybir.ActivationFunctionType

# module-level cache of the built program
_CACHE = {}

# set by run when trace requested (read by test.py)
LAST_EXEC_TIME_NS = None
LAST_RESULTS = None


def _grid_both(nc, tc, sb, psum, attx_dram, atty_dram, gx_sb, gy_sb,
               mid_hook=None):
    """Grid-gen for BOTH axes, x/y stacked in shared tiles so the serial
    dependency chain is paid once. Replicates the reference's XLA:CPU float
    behavior:
      - jnp.sum assoc: seq scan in 16 contiguous windows of 32, then seq
        over the 16 window sums.
      - jnp.cumsum assoc: blocked [32,16] scan (reduce-window lowering).
    All cross-partition movement goes through PE matmuls with 1.0 weights
    (identity / ones / block-indicator products, exact in fp32) or the PE
    transpose path (pure data movement).

    Engine plan: DVE owns the serial y-critical chain; Pool mirrors the
    x-axis work; ACT does the tent Abs/Relu ops and the x flatten copies;
    PE does all transposes/broadcast-feeding matmuls in y-then-x order so
    the main loop's stage-1 matmuls queue right behind gy.
    """
    f = FP32
    ones_col = sb.tile([1, P], f, tag="ones")
    nc.vector.memset(ones_col[:], 1.0)
    ones11 = ones_col[:, 0:1]
    id_sb = sb.tile([P, P], f, tag="id")
    make_identity(nc, id_sb[:])

    # warm the ACT function table before the critical path needs it
    warm = sb.tile([1, 1], f, tag="warm")
    nc.gpsimd.memset(warm[:], 0.0)
    nc.scalar.activation(warm[:], warm[:], ACT.Abs)
    nc.scalar.activation(warm[:], warm[:], ACT.Relu)

    trow = sb.tile([1, S], f, tag="trow")
    nc.gpsimd.iota(trow[:], pattern=[[1, S]], base=1, channel_multiplier=0,
                   allow_small_or_imprecise_dtypes=True)

    # ---- load att in both layouts, stacked (on the SP ring, queued ahead
    # of the channel loads). a16 is partition-padded: x windows on
    # partitions 0:16, y on 32:48, so matmul lhsT bases stay legal. -------
    a16 = sb.tile([64, 32], f, tag="a16")
    nc.gpsimd.memset(a16[:], 0.0)
    nc.sync.dma_start(out=a16[0:16, :],
                      in_=attx_dram.rearrange("(b i) -> b i", b=16))
    nc.sync.dma_start(out=a16[32:48, :],
                      in_=atty_dram.rearrange("(b i) -> b i", b=16))
    a32 = sb.tile([64, 16], f, tag="a32")
    nc.sync.dma_start(out=a32[0:32, :],
                      in_=attx_dram.rearrange("(r j) -> r j", r=32))
    nc.sync.dma_start(out=a32[32:64, :],
                      in_=atty_dram.rearrange("(r j) -> r j", r=32))

    def win_sums(t16, tag):
        """padded [64,32] (x rows 0:16 / y rows 32:48) -> SBUF [1,64] row
        of sequential window-sum prefixes; x total at free 15, y at 47."""
        pr = sb.tile([64, 32], f, tag="sumpr")
        nc.vector.tensor_tensor_scan(pr[:], t16[:], t16[:], 0.0, ALU.add,
                                     ALU.bypass)
        ps_w = psum.tile([1, 64], f, tag="gpsA")
        nc.tensor.matmul(ps_w[:], pr[:, 31:32], id_sb[0:64, 0:64],
                         start=True, stop=True)
        ws = sb.tile([1, 64], f, tag=tag)
        nc.vector.tensor_tensor_scan(ws[:, 0:16], ps_w[:, 0:16],
                                     id_sb[0:1, 0:16], 0.0, ALU.add,
                                     ALU.bypass)
        nc.vector.tensor_tensor_scan(ws[:, 32:48], ps_w[:, 32:48],
                                     id_sb[0:1, 0:16], 0.0, ALU.add,
                                     ALU.bypass)
        # totals as an adjacent [1,2] strided view (x at 15, y at 47)
        return ws[:, 15:48:32]

    def pair_bcast(tag, pair_row):
        """[1,2] (x,y) row -> [64,1] psum column via two accumulating
        mask-row outer products (exact: 1.0/0.0 weights)."""
        ps = psum.tile([64, 1], f, tag="gpsA")
        nc.tensor.matmul(ps[:], bx[tag[0:3]], pair_row[:, 0:1],
                         start=True, stop=False)
        nc.tensor.matmul(ps[:], by[tag[0:3]], pair_row[:, 1:2],
                         start=False, stop=True)
        return ps

    # mask rows for pair_bcast: out[m] = 1.0 on the axis' partition rows
    # b16: x rows 0:16 / y rows 32:48; b32: x rows 0:32 / y rows 32:64
    bx, by = {}, {}
    for name, (x0, x1, y0, y1) in (("b16", (0, 16, 32, 48)),
                                   ("b32", (0, 32, 32, 64))):
        mx = sb.tile([1, 64], f, tag=f"mx_{name}")
        nc.gpsimd.memset(mx[:], 0.0)
        nc.gpsimd.memset(mx[:, x0:x1], 1.0)
        my = sb.tile([1, 64], f, tag=f"my_{name}")
        nc.gpsimd.memset(my[:], 0.0)
        nc.gpsimd.memset(my[:, y0:y1], 1.0)
        bx[name], by[name] = mx[:], my[:]

    # ---- normalize: an = att / sum * S -----------------------------------
    sums = win_sums(a16, "ws_n")
    rrow = sb.tile([1, 2], f, tag="rrow")
    nc.vector.reciprocal(rrow[:], sums)
    r16_ps = pair_bcast("b16_r", rrow)
    r32_ps = pair_bcast("b32_r", rrow)
    an16 = sb.tile([64, 32], f, tag="an16")
    nc.vector.tensor_scalar(an16[:], a16[:], r16_ps[:], float(S),
                            op0=ALU.mult, op1=ALU.mult)
    an32 = sb.tile([64, 16], f, tag="an32")
    nc.vector.tensor_scalar(an32[:], a32[:], r32_ps[:], float(S),
                            op0=ALU.mult, op1=ALU.mult)

    # ---- 5 redistribute iterations ---------------------------------------
    # carried state: c16 = min(att, thr) (clipped layout for the sums) and
    # an32 = att (cumsum layout). Updates fuse to one op each, bitwise equal
    # to the reference's {clip; sum; redistribute} order. Ping-pong tags:
    # each update reads its previous tile (bufs=1 per tag).
    c16 = sb.tile([64, 32], f, tag="c16i")
    nc.vector.tensor_scalar(c16[:], an16[:], THR, None, op0=ALU.min)
    for it in range(ITERS):
        sums = win_sums(c16, "ws_i")
        drow = sb.tile([1, 2], f, tag="drow")
        nc.vector.tensor_scalar(drow[:], sums, -1.0 / S, 1.0,
                                op0=ALU.mult, op1=ALU.add)
        d16_ps = pair_bcast("b16_d", drow)
        d32_ps = pair_bcast("b32_d", drow)
        if it < ITERS - 1:
            c16n = sb.tile([64, 32], f, tag=f"c16{it % 2}")
            nc.vector.tensor_scalar(c16n[:], c16[:], d16_ps[:], THR,
                                    op0=ALU.add, op1=ALU.min)
            c16 = c16n
        an32n = sb.tile([64, 16], f, tag=f"an32{it % 2}")
        nc.vector.tensor_scalar(an32n[:], an32[:], THR, d32_ps[:],
                                op0=ALU.min, op1=ALU.add)
        an32 = an32n

    # ---- XLA cumsum replica (blocked [32,16] per axis, stacked [64,16]) --
    pr16 = sb.tile([64, 16], f, tag="pr16")
    nc.vector.tensor_tensor_scan(pr16[:], an32[:], an32[:], 0.0, ALU.add,
                                 ALU.bypass)
    ps_b = psum.tile([1, 64], f, tag="gpsA")
    nc.tensor.matmul(ps_b[:, 0:32], pr16[0:32, 15:16], id_sb[0:32, 0:32],
                     start=True, stop=True)
    nc.tensor.matmul(ps_b[:, 32:64], pr16[32:64, 15:16], id_sb[32:64, 32:64],
                     start=True, stop=True)
    # two-stage sequential scan over the 32 block sums per axis (XLA order)
    incl = sb.tile([1, 64], f, tag="incl")
    for base in (0, 32):
        nc.vector.tensor_tensor_scan(incl[:, base:base + 16],
                                     ps_b[:, base:base + 16],
                                     id_sb[0:1, 0:16], 0.0, ALU.add,
                                     ALU.bypass)
        nc.vector.tensor_tensor_scan(incl[:, base + 16:base + 32],
                                     ps_b[:, base + 16:base + 32],
                                     id_sb[0:1, 0:16], 0.0, ALU.add,
                                     ALU.bypass)
        nc.vector.tensor_scalar(incl[:, base + 16:base + 32],
                                incl[:, base + 16:base + 32],
                                incl[:, base + 15:base + 16], None,
                                op0=ALU.add)
    excl = sb.tile([1, 64], f, tag="excl")
    nc.vector.memset(excl[:], 0.0)
    nc.vector.tensor_copy(excl[:, 1:32], incl[:, 0:31])
    nc.vector.tensor_copy(excl[:, 33:64], incl[:, 32:63])
    ecol_ps = psum.tile([64, 1], f, tag="gpsA")
    nc.tensor.matmul(ecol_ps[:], excl[:], ones11, start=True, stop=True)
    csum2 = sb.tile([64, 16], f, tag="csum2")
    nc.vector.tensor_scalar(csum2[:], pr16[:], ecol_ps[:], None, op0=ALU.add)

    # ---- flatten csum [64,16] -> guarded row via PE transpose ------------
    # crow layout: [0]=0, x csum at 1..512, [513]=0, y csum at 514..1025.
    # c1row (csum shifted by one, zero-padded) is then just the off-by-one
    # view of the same buffer.
    ps_t = psum.tile([16, 64], f, tag="gpsA")
    nc.tensor.transpose(ps_t[:], csum2[:], id_sb[0:64, 0:64])
    # separate per-axis row tiles so the two engines' strided copies never
    # WAW-serialize against each other
    crow_x = sb.tile([1, S + 1], f, tag="crowx")
    crow_y = sb.tile([1, S + 1], f, tag="crowy")
    nc.gpsimd.memset(crow_x[0:1, 0:1], 0.0)
    nc.vector.memset(crow_y[0:1, 0:1], 0.0)
    vx = crow_x[0:1, 1:513].rearrange("a (r j) -> a r j", r=32)
    vy = crow_y[0:1, 1:513].rearrange("a (r j) -> a r j", r=32)
    for j in range(16):
        nc.vector.tensor_copy(vy[:, :, j], ps_t[j:j + 1, 32:64])
    for j in range(16):
        nc.scalar.copy(vx[:, :, j], ps_t[j:j + 1, 0:32])

    # per-axis phases; y first (stage-1 needs gy; mid_hook lets the caller
    # emit the first stage-1 channels into the PE queue before the x-phase
    # matmuls, so the main loop starts at gy-ready)
    ones_r = sb.tile([P, 1], F32R, tag="ones_r")
    nc.vector.memset(ones_r[:], 1.0)

    def axis_phase(g_sb, crow, veng, sfx):
        c1v = crow[0:1, 0:S]          # csum[s-1], zero-guarded
        cv = crow[0:1, 1:S + 1]       # csum[s]

        # rows: dd = max(csum - csum_sm1, 1e-8); rd = 1/dd
        dd = sb.tile([1, S], f, tag=f"dd{sfx}")
        veng.tensor_tensor(dd[:], cv, c1v, op=ALU.subtract)
        veng.tensor_scalar(dd[:], dd[:], 1e-8, None, op0=ALU.max)
        rd = sb.tile([1, S], f, tag=f"rd{sfx}")
        nc.vector.reciprocal(rd[:], dd[:])

        # targets: tgt = step * (1..512), step = csum[-1]/512, broadcast
        stp = sb.tile([1, 1], f, tag=f"stp{sfx}")
        veng.tensor_scalar(stp[:], crow[0:1, S:S + 1],
                           1.0 / S, None, op0=ALU.mult)
        tgt = sb.tile([1, S], f, tag=f"tgt{sfx}")
        veng.tensor_scalar(tgt[:], trow[:], stp[:], None, op0=ALU.mult)
        tgtb = sb.tile([P, S], f, tag=f"tgtb{sfx}")
        nc.gpsimd.partition_broadcast(tgtb[:], tgt[:])

        # csum_sm1 / rd as per-partition scalar columns [128, 2*NCH]
        # (s = 128k + p), via tiny exact ones-product matmuls
        gcol_ps = psum.tile([P, 2 * NCH], f, tag="gpsA")
        for k in range(NCH):
            nc.tensor.matmul(gcol_ps[:, k:k + 1],
                             c1v[:, k * P:(k + 1) * P], ones11,
                             start=True, stop=True)
            nc.tensor.matmul(gcol_ps[:, NCH + k:NCH + k + 1],
                             rd[:, k * P:(k + 1) * P], ones11,
                             start=True, stop=True)
        gcol = sb.tile([P, 2 * NCH], f, tag=f"gcol{sfx}")
        veng.tensor_copy(gcol[:], gcol_ps[:])

        # p row = sum_s clip((tgt[t]-csum_sm1[s])*rd[s], 0, 1) = j + frac,
        # accumulated over the 4 s-chunks by exact ones-matmuls on PE
        prow_ps = psum.tile([1, S], f, tag="gpsR")
        for k in range(NCH):
            t2 = sb.tile([P, S], F32R, tag=f"pt{sfx}{k % 2}")
            veng.tensor_scalar(t2[:], tgtb[:], gcol[:, k:k + 1],
                               gcol[:, NCH + k:NCH + k + 1],
                               op0=ALU.subtract, op1=ALU.mult)
            veng.tensor_scalar(t2[:], t2[:], 0.0, 1.0, op0=ALU.max,
                               op1=ALU.min)
            nc.tensor.matmul(prow_ps[:], ones_r[:], t2[:],
                             start=(k == 0), stop=(k == NCH - 1))

        # p_img = p * 511/512 (matches the reference's coord chain to ~1
        # ulp; the inverse-CDF p is scale-exact either way), broadcast
        p_img = sb.tile([1, S], f, tag=f"pimg{sfx}")
        veng.tensor_scalar(p_img[:], prow_ps[:], float(S - 1) / S, None,
                           op0=ALU.mult)
        pb = sb.tile([P, S], f, tag=f"pb{sfx}")
        nc.gpsimd.partition_broadcast(pb[:], p_img[:])

        # tent build: G[s,t] = relu(1 - |p[t] - s|), bitwise equal to the
        # reference's clip(p-s+1,0,1)-clip(p-s,0,1) for the 2-nonzero rows.
        # gx in blocked layout s = 128k+p (stage-2 k-chunks);
        # gy in interleaved layout s = 4p+q (8KB-descriptor loads)
        for k in range(NCH):
            scol = sb.tile([P, 1], f, tag=f"scol{sfx}{k % 2}")
            if g_sb is gx_sb:
                nc.gpsimd.iota(scol[:], pattern=[[0, 1]], base=k * P,
                               channel_multiplier=1,
                               allow_small_or_imprecise_dtypes=True)
            else:
                nc.gpsimd.iota(scol[:], pattern=[[0, 1]], base=k,
                               channel_multiplier=NCH,
                               allow_small_or_imprecise_dtypes=True)
            teng = veng
            t0 = sb.tile([P, S], f, tag=f"g0{sfx}{k % 2}")
            teng.tensor_scalar(t0[:], pb[:], scol[:], None, op0=ALU.subtract)
            ab = sb.tile([P, S], f, tag=f"ga{sfx}{k % 2}")
            if teng is nc.vector:
                teng.scalar_tensor_tensor(ab[:], t0[:], -1.0, t0[:],
                                          op0=ALU.mult, op1=ALU.max)
            else:
                nc.scalar.activation(ab[:], t0[:], ACT.Abs)
            nc.scalar.activation(g_sb[:, k, :], ab[:], ACT.Relu, bias=1.0,
                                 scale=-1.0)

    axis_phase(gy_sb, crow_y, nc.vector, "y")
    if mid_hook is not None:
        mid_hook()
    axis_phase(gx_sb, crow_x, nc.gpsimd, "x")

    return {"crow_x": crow_x[:], "crow_y": crow_y[:]}


def _build_program():
    nc = bacc.Bacc("TRN2", target_bir_lowering=False, debug=False,
                   num_devices=N_CORES)

    data_h = nc.dram_tensor("data", [C, S, S], FP32, kind="ExternalInput")
    attx_h = nc.dram_tensor("attx", [S], FP32, kind="ExternalInput")
    atty_h = nc.dram_tensor("atty", [S], FP32, kind="ExternalInput")
    out_h = nc.dram_tensor("out", [C, S, S], FP32, kind="ExternalOutput")

    with tile.TileContext(nc) as tc:
        from contextlib import ExitStack
        with ExitStack() as ctx:
            # long-lived pools FIRST so grid temporaries never alias them
            # (aliasing would stall the prefetched loads on grid-gen)
            gpool = ctx.enter_context(tc.tile_pool(name="g_sb", bufs=1))
            gx_sb = gpool.tile([P, NCH, S], F32R, tag="gx")
            gy_sb = gpool.tile([P, NCH, S], F32R, tag="gy")

            dpool = ctx.enter_context(tc.tile_pool(name="dtile", bufs=6))
            tpool = ctx.enter_context(tc.tile_pool(name="t1t", bufs=4))
            opool = ctx.enter_context(tc.tile_pool(name="osb", bufs=3))
            ps1 = ctx.enter_context(
                tc.tile_pool(name="ps1", bufs=3, space="PSUM"))
            ps2 = ctx.enter_context(
                tc.tile_pool(name="ps2", bufs=3, space="PSUM"))

            PIPE = 3  # stage-1 lookahead channels emitted before the x-grid

            def emit_stage1(c):
                # interleaved row layout: partition p holds rows 4p..4p+3
                # (one contiguous 8KB descriptor per partition)
                dt = dpool.tile([P, NCH, S], F32R, tag="d")
                nc.sync.dma_start(
                    out=dt[:],
                    in_=data_h[c].rearrange("(p q) w -> p q w", p=P).bitcast(F32R))
                # stage 1: t1T[w, oh] = sum_h data[h, w] * GyT[h, oh],
                # contraction split by q = h%4 (gy is s=4p+q interleaved)
                t1 = tpool.tile([P, NCH, S], F32R, tag="t1")
                for m in range(NCH):
                    pt = ps1.tile([P, S], FP32, tag="ps1")
                    for q in range(NCH):
                        nc.tensor.matmul(pt[:],
                                         dt[:, q, m * P:(m + 1) * P],
                                         gy_sb[:, q, :],
                                         start=(q == 0), stop=(q == NCH - 1))
                    nc.vector.tensor_copy(t1[:, m, :], pt[:])
                return t1

            def emit_stage2(c, t1):
                # stage 2: out[oh, ow] = sum_w t1T[w, oh] * GxT[w, ow];
                # m-groups pick oh = 4p+q (stride-4 slice) so each q-group
                # store is one contiguous 2KB descriptor per partition
                osb = opool.tile([P, NCH, S], FP32, tag="o")
                ov = out_h[c].rearrange("(p q) w -> p q w", p=P)
                for q in range(NCH):
                    pt = ps2.tile([P, S], FP32, tag="ps2")
                    for k in range(NCH):
                        nc.tensor.matmul(pt[:],
                                         t1[:, k, q::NCH],
                                         gx_sb[:, k, :],
                                         start=(k == 0), stop=(k == NCH - 1))
                    nc.scalar.copy(osb[:, q, :], pt[:])
                    # store on the SP HWDGE ring: loads always run ahead of
                    # the matching store there, and keeping stores off the
                    # ACT ring stops them serializing against the psum
                    # copies; per-q stores shrink the drain tail
                    nc.sync.dma_start(out=ov[:, q, :], in_=osb[:, q, :])

            t1s = {}

            def mid_hook():
                for c in range(PIPE):
                    t1s[c] = emit_stage1(c)

            with ExitStack() as gctx:
                sb = gctx.enter_context(tc.tile_pool(name="grid_sb", bufs=1))
                psum_g = gctx.enter_context(
                    tc.tile_pool(name="grid_ps", bufs=1, space="PSUM"))
                dbg = _grid_both(nc, tc, sb, psum_g, attx_h[:], atty_h[:],
                                 gx_sb, gy_sb, mid_hook=mid_hook)
                if os.environ.get("ATT_DEBUG_GRID"):
                    dbg_gy = nc.dram_tensor("dbg_gy", [P, NCH, S], FP32,
                                            kind="ExternalOutput")
                    dbg_gx = nc.dram_tensor("dbg_gx", [P, NCH, S], FP32,
                                            kind="ExternalOutput")
                    dbg_crow = nc.dram_tensor("dbg_crow", [2 * S + 2], FP32,
                                              kind="ExternalOutput")
                    nc.scalar.dma_start(out=dbg_gy[:],
                                        in_=gy_sb[:].bitcast(FP32))
                    nc.scalar.dma_start(out=dbg_gx[:],
                                        in_=gx_sb[:].bitcast(FP32))
                    nc.scalar.dma_start(
                        out=dbg_crow[0:S + 1].rearrange("(a s) -> a s", a=1),
                        in_=dbg["crow_x"])
                    nc.scalar.dma_start(
                        out=dbg_crow[S + 1:2 * S + 2].rearrange(
                            "(a s) -> a s", a=1),
                        in_=dbg["crow_y"])

            for c in range(C):
                if c + PIPE < C:
                    t1s[c + PIPE] = emit_stage1(c + PIPE)
                emit_stage2(c, t1s.pop(c))

    nc.compile()
    return nc


def _get_program():
    key = "nc"
    if key not in _CACHE:
        _CACHE[key] = _build_program()
    return _CACHE[key]


def kernel(data, attx, atty):
    global LAST_EXEC_TIME_NS, LAST_RESULTS
    data = np.ascontiguousarray(data, dtype=np.float32)
    attx = np.ascontiguousarray(attx, dtype=np.float32)
    atty = np.ascontiguousarray(atty, dtype=np.float32)
    N = data.shape[0]
    assert N == N_CORES

    nc = _get_program()
    in_maps = [
        {
            "data": data[i],
            "attx": attx[i].reshape(S),
            "atty": atty[i].reshape(S),
        }
        for i in range(N)
    ]
    trace = bool(int(os.environ.get("ATT_KERNEL_TRACE", "0")))
    try:
        res = run_bass_kernel_spmd(nc, in_maps, list(range(N_CORES)),
                                   trace=trace)
    except ModuleNotFoundError:
        # NTFF profile hook unavailable in this environment
        res = run_bass_kernel_spmd(nc, in_maps, list(range(N_CORES)),
                                   trace=False)
    LAST_EXEC_TIME_NS = res.exec_time_ns
    LAST_RESULTS = res
    out = np.stack([res.results[i]["out"] for i in range(N)], axis=0)
    return out
